# revision 29
# baseline (speedup 1.0000x reference)
"""Fused LayerNorm + multi-head attention Trainium2 kernel, 8-core SPMD.

Problem: x[4, 2048, 768] -> LN -> QKV (w_qkv[2304, 768]) -> 12-head attention
         -> out proj (w_out[768, 768] + b_out). f32 I/O, bf16 tensor-engine compute.

Sharding: core c handles batch b=c//2, query-half g=c%2 (1024 queries each).
Each core receives the FULL (rotated) sequence of its batch so K/V are computed
locally -- no collectives. The token order is rotated per-core so the core's own
query chunk is always columns [0, 1024) => identical SPMD program on all cores.

v2: the attention loop is software-pipelined so the ACT engine (exp) never
starves. Per (head-pair hp, query-half qh, kv-tile jt): one [128,1024] PSUM
scores tile holds both heads of the pair (h0 cols 0:512, h1 cols 512:1024),
one exp call covers both, and the two scores matmuls use disjoint PE row
groups (partitions 0:64 vs 64:128) so they stream concurrently. AV
accumulates into a single U [128,1024] PSUM tile per (hp, qh); V carries an
appended ones column so U row 64 is the softmax denominator. Denominators are
staged to SBUF during the loop; reciprocal (Ln+Exp), PE-matmul broadcast and
DVE normalize run batched at the tail. QKV matmuls not needed to start
attention are dribbled into the loop as PE filler. LayerNorm stats run in
bf16, with mean/bias folded into the QKV matmul via two appended rows.
"""

import numpy as np
import ml_dtypes

import concourse.bass as bass
import concourse.tile as tile
from concourse import bacc, mybir
from concourse.bass_utils import run_bass_kernel_spmd

F32 = mybir.dt.float32
BF16 = mybir.dt.bfloat16
AF = mybir.ActivationFunctionType
ALU = mybir.AluOpType

DIM = 768
HEADS = 12
B, N = 4, 2048
D = 64          # head dim
NQ = 1024       # queries per core
CT = 6          # 768 / 128 channel tiles
NT = 16         # 2048 / 128 token tiles
HP = 6          # head pairs

LAST = None  # BassKernelResults of the most recent run (for test harness)
_NC = None


def build():
    nc = bacc.Bacc("TRN2", target_bir_lowering=False, debug=False, num_devices=8)

    xT = nc.dram_tensor("xT", [DIM, N], BF16, kind="ExternalInput")
    wqkvT = nc.dram_tensor("wqkvT", [DIM + 2, 3 * DIM], BF16, kind="ExternalInput")
    woutT = nc.dram_tensor("woutT", [DIM + 1, DIM], BF16, kind="ExternalInput")
    selIn = nc.dram_tensor("selIn", [HEADS, HEADS * D], BF16, kind="ExternalInput")
    outT = nc.dram_tensor("outT", [DIM, NQ], F32, kind="ExternalOutput")
    import os
    dbg = os.environ.get("KDEBUG", "0") == "1"
    if dbg:
        d_xt = nc.dram_tensor("d_xt", [128, CT, N], BF16, kind="ExternalOutput")
        d_KT = nc.dram_tensor("d_KT", [128, CT, N], BF16, kind="ExternalOutput")
        d_QT = nc.dram_tensor("d_QT", [128, CT, NQ], BF16, kind="ExternalOutput")
        d_V4 = nc.dram_tensor("d_V4", [128, NT, HEADS, D + 1], BF16, kind="ExternalOutput")
        d_AO = nc.dram_tensor("d_AO", [128, HP, NQ], BF16, kind="ExternalOutput")
        d_den = nc.dram_tensor("d_den", [HEADS, NQ], BF16, kind="ExternalOutput")
        d_rb = nc.dram_tensor("d_rb", [HEADS, NQ], BF16, kind="ExternalOutput")

    with tile.TileContext(nc) as tc:
        with (
            tc.tile_pool(name="persist", bufs=1) as P1,
            tc.tile_pool(name="work", bufs=2) as PW,
            tc.tile_pool(name="et", bufs=3) as PET,
            tc.tile_pool(name="ps", bufs=1, space="PSUM") as PS,
        ):
            # ---- persistent SBUF tensors ----
            wq = P1.tile([128, CT, 3 * DIM], BF16)       # W'' rows 0..767
            wex = P1.tile([2, 3 * DIM], BF16)            # W'' rows 768..769
            WO = P1.tile([128, CT, DIM], BF16)           # w_out^T  (f-major tiles)
            wob = P1.tile([1, DIM], BF16)                # b_out row
            xt = P1.tile([128, CT, N], BF16, tag="big_a")  # bf16 x, then x~=x*rstd
            xex = P1.tile([2, N], BF16)                  # x~ rows 768 (-mu*rstd), 769 (1)
            KT = P1.tile([128, CT, N], BF16)             # K^T channel-major
            QT = P1.tile([128, CT, NQ], BF16)            # Q^T channel-major
            V4 = P1.tile([128, NT, HEADS, D + 1], BF16)  # V token-major + ones col
            AO = P1.tile([128, HP, NQ], BF16)            # attention out, f-major
            ones1b = P1.tile([128, 1], BF16)             # ones column (stats lhsT)
            sel = P1.tile([HEADS, HEADS, D], BF16)       # one-hot rows: sel[k,k,:]=1
            onesr = P1.tile([1, 128], BF16)              # ones row (warmup lhsT)
            onesrs = P1.tile([1, 128], BF16)             # sqrt(768) row (rstd bcast)
            onesI = P1.tile([1, NQ], BF16)               # ones row (bias rhs)
            epsc = P1.tile([1, 1], F32)
            lnDc = P1.tile([1, 1], F32)
            r_a = P1.tile([1, N], F32)                   # E[x^2] -> var -> rstd
            r_b = P1.tile([1, N], F32)                   # mu -> mu*rstd
            r_ab = P1.tile([1, N], BF16)                 # rstd bf16 (bcast rhs)
            rb_sb = P1.tile([128, 2, NQ], BF16)          # rstd bcast to 128 parts/half
            den_all = P1.tile([HEADS, NQ], BF16)          # softmax denominators
            den_st = P1.tile([65, NQ], BF16)              # staging (row 64 used)
            rb_all = P1.tile([HEADS, NQ], BF16)          # 1/den bf16

            nc.vector.memset(epsc[:], float(DIM) * 1e-5)
            nc.vector.memset(ones1b[:], 1.0)
            nc.vector.memset(onesr[:], 1.0)
            nc.vector.memset(onesrs[:], 1.0)
            nc.vector.memset(lnDc[:], 0.5 * float(np.log(DIM)))
            nc.vector.memset(onesI[:], 1.0)
            # row 1 must stay 1.0; row 0 is overwritten with -mu*rstd below
            nc.vector.memset(xex[:, :], 1.0)
            nc.vector.memset(V4[:, :, :, D : D + 1], 1.0)

            # ---- PE warmup spin + ACT table preload (runs during the x DMA;
            # keeps HAM at K=8/8 so head/early-D matmuls run at full clock) ----
            warm = PS.tile([128, 512], F32, tag="fill", bufs=2, name="warm")
            for i in range(26):
                nc.tensor.matmul(
                    warm[:], onesr[:], onesI[:, 0:512],
                    start=(i == 0), stop=(i == 25),
                )
            nc.scalar.activation(epsc[:], epsc[:], AF.Ln, bias=1.0)
            nc.vector.memset(epsc[:], float(DIM) * 1e-5)

            # ---- DMAs: x first (gates everything), weights on the gpsimd queue ----
            for h in range(2):
                for ct in range(CT):
                    nc.sync.dma_start(
                        xt[:, ct, h * 1024 : (h + 1) * 1024],
                        xT[ct * 128 : (ct + 1) * 128, h * 1024 : (h + 1) * 1024],
                    )
            for ct in range(CT):
                nc.gpsimd.dma_start(wq[:, ct, :], wqkvT[ct * 128 : (ct + 1) * 128, :])
            nc.gpsimd.dma_start(wex[:], wqkvT[DIM : DIM + 2, :])
            for ct in range(CT):
                nc.gpsimd.dma_start(WO[:, ct, :], woutT[ct * 128 : (ct + 1) * 128, :])
            nc.gpsimd.dma_start(wob[:], woutT[DIM : DIM + 1, :])
            nc.gpsimd.dma_start(sel[:].rearrange("h k d -> h (k d)"), selIn[:, :])

            # ---- phase A: per-half LN stats (bf16) + rstd + x~ ----
            for h in range(2):
                hsl = slice(h * 1024, (h + 1) * 1024)
                sts = PS.tile([128, 1024], F32, tag="sp", bufs=2, name=f"sts_{h}")
                for ct in range(CT):
                    xsq = PW.tile([128, 1024], BF16, tag="xsq", name=f"xsq_{h}_{ct}")
                    nc.vector.tensor_tensor(
                        xsq[:], xt[:, ct, hsl], xt[:, ct, hsl], ALU.mult
                    )
                    for s in range(2):
                        osl = slice(s * 512, (s + 1) * 512)
                        csl = slice(h * 1024 + s * 512, h * 1024 + (s + 1) * 512)
                        nc.tensor.matmul(
                            sts[0:1, osl], ones1b[:], xt[:, ct, csl],
                            start=(ct == 0), stop=(ct == CT - 1),
                        )
                        nc.tensor.matmul(
                            sts[64:65, osl], ones1b[:], xsq[:, osl],
                            start=(ct == 0), stop=(ct == CT - 1),
                        )
                # u = S1^2/768 (ACT), t = S2 - u (DVE), then
                # rstd/sqrt(768) = exp(-0.5*ln(t + 768*eps)); the sqrt(768)
                # factor is folded into the broadcast ones row (onesrs).
                nc.scalar.activation(
                    r_b[:, hsl], sts[0:1, :], AF.Square, scale=DIM ** -0.5
                )
                nc.vector.tensor_tensor(
                    r_a[:, hsl], sts[64:65, :], r_b[:, hsl], ALU.subtract
                )
                nc.scalar.activation(r_a[:, hsl], r_a[:, hsl], AF.Ln, bias=epsc[:])
                nc.scalar.activation(
                    r_a[:, hsl], r_a[:, hsl], AF.Exp, scale=-0.5, bias=lnDc[:]
                )
                nc.vector.tensor_copy(r_ab[:, hsl], r_a[:, hsl])    # bf16 for bcast MM
                # broadcast rstd over 128 partitions via PE, stage to SBUF bf16
                rb_ps = PS.tile([128, 1024], F32, tag="U", bufs=1, name=f"rb_ps_{h}")
                for s in range(2):
                    nc.tensor.matmul(
                        rb_ps[:, s * 512 : (s + 1) * 512],
                        onesrs[:],
                        r_ab[:, h * 1024 + s * 512 : h * 1024 + (s + 1) * 512],
                    )
                nc.vector.tensor_copy(rb_sb[:, h, :], rb_ps[:])
                # x~ = x * rstd in place (bf16 * bf16)
                for ct in range(CT):
                    nc.vector.tensor_tensor(
                        xt[:, ct, hsl], xt[:, ct, hsl], rb_sb[:, h, :], ALU.mult
                    )
                # folded-LN extra row: -mu*rstd = -(S1 * r_a)/sqrt(768)
                nc.vector.tensor_tensor(r_b[:, hsl], sts[0:1, :], r_a[:, hsl], ALU.mult)
                nc.vector.tensor_scalar_mul(xex[0:1, hsl], r_b[:, hsl], -1.0 / DIM)

            # ---- filler work queue ----
            def kq_unit(which, fidx, nh, s, scalar_copy=False):
                base = DIM if which == "K" else 0
                fsl = slice(base + fidx * 128, base + (fidx + 1) * 128)
                n0 = nh * 1024 + s * 512
                dst = (KT if which == "K" else QT)[:, fidx, n0 : n0 + 512]

                def go():
                    acc = PS.tile(
                        [128, 512], F32, tag="fill", bufs=2,
                        name=f"kq{which}_{fidx}_{nh}_{s}",
                    )
                    for ct in range(CT + 1):
                        yield nc.tensor.matmul(
                            acc[:],
                            wq[:, ct, fsl] if ct < CT else wex[:, fsl],
                            xt[:, ct, n0 : n0 + 512] if ct < CT else xex[:, n0 : n0 + 512],
                            start=(ct == 0), stop=(ct == CT),
                        )
                    if scalar_copy:
                        nc.scalar.copy(dst, acc[:])
                    else:
                        nc.vector.tensor_copy(dst, acc[:])
                return go

            def v_unit(nt, lo, sz, scalar_copy=False):
                nsl = slice(nt * 128, (nt + 1) * 128)
                h0 = lo // 64  # first head covered

                def go():
                    acc = PS.tile(
                        [128, 512], F32, tag="fill", bufs=2, name=f"v_{nt}_{lo}"
                    )
                    for ct in range(CT + 1):
                        fsl = slice(2 * DIM + lo, 2 * DIM + lo + sz)
                        yield nc.tensor.matmul(
                            acc[:, 0:sz],
                            xt[:, ct, nsl] if ct < CT else xex[:, nsl],
                            wq[:, ct, fsl] if ct < CT else wex[:, fsl],
                            start=(ct == 0), stop=(ct == CT),
                        )
                    cp = nc.scalar.copy if scalar_copy else nc.vector.tensor_copy
                    cp(
                        V4[:, nt, h0 : h0 + sz // 64, 0:D],
                        acc[:, 0:sz].rearrange("p (h d) -> p h d", d=D),
                    )
                return go

            class Filler:
                """Work queue of matmul-chain generators, each with a deadline
                (iteration index). emit(i) advances the queue by the normal
                quota but ALWAYS finishes every unit whose deadline is <= i:
                a unit's instructions must be emitted in program order before
                the attention instruction that reads its output, or the
                dependency is silently missed (read-before-write)."""

                def __init__(self):
                    self.units = []   # (deadline, generator-fn)
                    self.cur = None
                    self.cur_deadline = None

                def add(self, deadline, go):
                    self.units.append((deadline, go))

                def emit(self, i, quota):
                    while True:
                        if self.cur is None:
                            if not self.units:
                                return
                            if quota <= 0 and self.units[0][0] > i:
                                return
                            self.cur_deadline, go = self.units.pop(0)
                            self.cur = go()
                        if quota <= 0 and self.cur_deadline > i:
                            return
                        try:
                            next(self.cur)
                            quota -= 1
                        except StopIteration:
                            self.cur = None

                def drain(self):
                    self.emit(10 ** 9, 10 ** 9)

            fill = Filler()

            # head: K/Q pair 0 over the local queries (jt 0-3) + V tile 0
            for go in (
                [kq_unit("K", 0, 0, s) for s in range(2)]
                + [kq_unit("Q", 0, 0, s) for s in range(2)]
                + [v_unit(nt, 0, 512) for nt in range(NT)]
                + [kq_unit("K", 1, 0, s) for s in range(2)]
                + [kq_unit("Q", 1, 0, s) for s in range(2)]
            ):
                for _ in go():
                    pass

            # filler with deadlines (iteration index in the 192-iter space):
            #   V-512 tile j feeds AV at iter j (and 16+j); K(p,nh,s) feeds
            #   scores at iter 32p + nh*8 + s*4 (issued one iter early);
            #   Q(p,s) feeds qh=s pass of pair p; V-256 tile j feeds hp4.
            fill.add(6, kq_unit("K", 0, 1, 0))
            fill.add(10, kq_unit("K", 0, 1, 1))

            for p in range(1, HP):
                d = 32 * p - 2
                if p > 1:
                    fill.add(d, kq_unit("K", p, 0, 0))
                    fill.add(d, kq_unit("Q", p, 0, 0))
                    fill.add(d + 2, kq_unit("K", p, 0, 1))
                fill.add(d + 8, kq_unit("K", p, 1, 0))
                fill.add(d + 10, kq_unit("K", p, 1, 1))
                if p > 1:
                    fill.add(d + 14, kq_unit("Q", p, 0, 1))
                if p == 2:
                    for nt in range(NT):
                        fill.add(64 + 4 * nt, v_unit(nt, 512, 256))

            # ---- phase D: attention, software-pipelined ----
            scale = float(D) ** -0.5
            iters = [
                (hp, qh, jt) for hp in range(HP) for qh in range(2) for jt in range(NT)
            ]
            sp_tiles = {}

            def issue_scores(hp, qh, jt):
                sp = PS.tile(
                    [128, 1024], F32, tag="sp", bufs=2, name=f"sp_{hp}_{qh}_{jt}"
                )
                sp_tiles[(hp, qh, jt)] = sp
                jsl = slice(jt * 128, (jt + 1) * 128)
                qsl = slice(qh * 512, (qh + 1) * 512)
                # h0 rows 0:64, h1 rows 64:128 -> adjacent MMs stream concurrently
                nc.tensor.matmul(
                    sp[:, 0:512], KT[0:64, hp, jsl], QT[0:64, hp, qsl],
                    start=True, stop=True,
                )
                nc.tensor.matmul(
                    sp[:, 512:1024], KT[64:128, hp, jsl], QT[64:128, hp, qsl],
                    start=True, stop=True,
                )

            fill.units.sort(key=lambda u: u[0])
            issue_scores(*iters[0])
            U = None
            for idx, (hp, qh, jt) in enumerate(iters):
                if idx + 1 < len(iters):
                    issue_scores(*iters[idx + 1])
                sp = sp_tiles.pop((hp, qh, jt))
                ET = PET.tile([128, 1024], BF16, tag="et", name=f"et_{hp}_{qh}_{jt}")
                nc.scalar.activation(ET[:], sp[:], AF.Exp, scale=scale)
                if jt == 0:
                    U = PS.tile([128, 1024], F32, tag="U", bufs=1, name=f"U_{hp}_{qh}")
                nc.tensor.matmul(
                    U[0 : D + 1, 0:512], V4[:, jt, 2 * hp, :], ET[:, 0:512],
                    start=(jt == 0), stop=(jt == NT - 1),
                )
                nc.tensor.matmul(
                    U[0 : D + 1, 512:1024], V4[:, jt, 2 * hp + 1, :], ET[:, 512:1024],
                    start=(jt == 0), stop=(jt == NT - 1),
                )
                fill.emit(idx, 2)
                if jt == NT - 1:
                    # drain U: raw AV to AO (bf16), denominators to SBUF staging
                    qsl = slice(qh * 512, (qh + 1) * 512)
                    nc.vector.tensor_copy(AO[0:64, hp, qsl], U[0:64, 0:512])
                    AOtmp = PW.tile(
                        [64, 512], BF16, tag="AOtmp", name=f"AOtmp_{hp}_{qh}"
                    )
                    nc.vector.tensor_copy(AOtmp[:], U[0:64, 512:1024])
                    nc.sync.dma_start(AO[64:128, hp, qsl], AOtmp[:])
                    nc.vector.tensor_copy(den_st[64:65, :], U[64:65, :])
                    nc.sync.dma_start(
                        den_all[2 * hp : 2 * hp + 1, qsl], den_st[64:65, 0:512]
                    )
                    nc.sync.dma_start(
                        den_all[2 * hp + 1 : 2 * hp + 2, qsl], den_st[64:65, 512:1024]
                    )
            fill.drain()

            # ---- tail: reciprocal, broadcast, normalize, out-projection ----
            # out-proj bias matmuls first: dependency-free PE work that keeps
            # the HAM warm while the reciprocal chain (2 table loads + Ln+Exp)
            # runs on ACT.
            po_tiles = {}
            for ot in range(2):
                osl = slice(ot * 128, (ot + 1) * 128)
                po = PS.tile([128, 1024], F32, tag="sp", bufs=2, name=f"po_{ot}")
                po_tiles[ot] = po
                for s in range(2):
                    ssl = slice(s * 512, (s + 1) * 512)
                    nc.tensor.matmul(
                        po[:, ssl], wob[:, osl], onesI[:, ssl],
                        start=True, stop=False,
                    )
            lnd_ps = PS.tile([128, 1024], F32, tag="U", bufs=1, name="lnd_ps")
            nc.scalar.activation(lnd_ps[0:HEADS, :], den_all[:], AF.Ln)
            nc.scalar.activation(rb_all[:], lnd_ps[0:HEADS, :], AF.Exp, scale=-1.0)
            for hp in range(HP):
                # broadcast 1/den rows over 64 partitions via one-hot selector:
                # rbB[0:64] = sel[:,2hp,:].T @ rb_all[0:12], rbB[64:128] likewise
                rbB = PS.tile([128, 1024], F32, tag="U", bufs=1, name=f"rbB_{hp}")
                for s in range(2):
                    ssl = slice(s * 512, (s + 1) * 512)
                    nc.tensor.matmul(
                        rbB[0:64, ssl], sel[:, 2 * hp, :], rb_all[:, ssl],
                    )
                    nc.tensor.matmul(
                        rbB[64:128, ssl], sel[:, 2 * hp + 1, :], rb_all[:, ssl],
                    )
                nc.vector.tensor_tensor(AO[:, hp, :], AO[:, hp, :], rbB[:], ALU.mult)

            if dbg:
                nc.sync.dma_start(d_xt[:], xt[:])
                nc.sync.dma_start(d_KT[:], KT[:])
                nc.sync.dma_start(d_QT[:], QT[:])
                nc.sync.dma_start(d_V4[:], V4[:])
                nc.sync.dma_start(d_AO[:], AO[:])
                nc.sync.dma_start(d_den[:], den_all[:])
                nc.sync.dma_start(d_rb[:], rb_all[:])

            for ot in range(CT):
                osl = slice(ot * 128, (ot + 1) * 128)
                if ot in po_tiles:
                    po = po_tiles[ot]
                else:
                    po = PS.tile([128, 1024], F32, tag="sp", bufs=2, name=f"po_{ot}")
                for s in range(2):
                    ssl = slice(s * 512, (s + 1) * 512)
                    if ot not in po_tiles:
                        nc.tensor.matmul(
                            po[:, ssl], wob[:, osl], onesI[:, ssl],
                            start=True, stop=False,
                        )
                    for ft in range(CT):
                        nc.tensor.matmul(
                            po[:, ssl], WO[:, ft, osl], AO[:, ft, ssl],
                            start=False, stop=(ft == CT - 1),
                        )
                outsb = PW.tile([128, 1024], F32, tag="outsb", bufs=1, name=f"outsb_{ot}")
                nc.scalar.copy(outsb[:], po[:])
                nc.sync.dma_start(outT[osl, :], outsb[:])

    nc.finalize()
    return nc


def _get_nc():
    global _NC
    if _NC is None:
        _NC = build()
    return _NC


def kernel(x, ln_w, ln_b, w_qkv, w_out, b_out):
    global LAST
    x = np.asarray(x, dtype=np.float32)
    ln_w = np.asarray(ln_w, dtype=np.float32)
    ln_b = np.asarray(ln_b, dtype=np.float32)
    w_qkv = np.asarray(w_qkv, dtype=np.float32)
    w_out = np.asarray(w_out, dtype=np.float32)
    b_out = np.asarray(b_out, dtype=np.float32)

    bf16 = ml_dtypes.bfloat16
    # W'' = [ (w_qkv * ln_w)^T ; rowsum of (w_qkv*ln_w) ; w_qkv @ ln_b ]
    wprime = w_qkv * ln_w[None, :]
    wqkvT = np.concatenate(
        [wprime.T, wprime.sum(axis=1)[None, :], (w_qkv @ ln_b)[None, :]], axis=0
    ).astype(bf16)
    woutT = np.concatenate([w_out.T, b_out[None, :]], axis=0).astype(bf16)
    selmat = np.kron(np.eye(HEADS, dtype=np.float32), np.ones((1, D), np.float32)).astype(bf16)

    in_maps = []
    for c in range(8):
        b, g = c // 2, c % 2
        order = np.r_[g * NQ : (g + 1) * NQ, (1 - g) * NQ : (2 - g) * NQ]
        xTc = np.ascontiguousarray(x[b][order].T).astype(bf16)
        in_maps.append({"xT": xTc, "wqkvT": wqkvT, "woutT": woutT, "selIn": selmat})

    nc = _get_nc()
    LAST = run_bass_kernel_spmd(nc, in_maps, core_ids=list(range(8)))

    out = np.empty((B, N, DIM), dtype=np.float32)
    for c in range(8):
        b, g = c // 2, c % 2
        out[b, g * NQ : (g + 1) * NQ, :] = LAST.results[c]["outT"].T
    return out


# revision 30
# speedup vs baseline: 1.0045x; 1.0045x over previous
"""Fused LayerNorm + multi-head attention Trainium2 kernel, 8-core SPMD.

Problem: x[4, 2048, 768] -> LN -> QKV (w_qkv[2304, 768]) -> 12-head attention
         -> out proj (w_out[768, 768] + b_out). f32 I/O, bf16 tensor-engine compute.

Sharding: core c handles batch b=c//2, query-half g=c%2 (1024 queries each).
Each core receives the FULL (rotated) sequence of its batch so K/V are computed
locally -- no collectives. The token order is rotated per-core so the core's own
query chunk is always columns [0, 1024) => identical SPMD program on all cores.

v2: the attention loop is software-pipelined so the ACT engine (exp) never
starves. Per (head-pair hp, query-half qh, kv-tile jt): one [128,1024] PSUM
scores tile holds both heads of the pair (h0 cols 0:512, h1 cols 512:1024),
one exp call covers both, and the two scores matmuls use disjoint PE row
groups (partitions 0:64 vs 64:128) so they stream concurrently. AV
accumulates into a single U [128,1024] PSUM tile per (hp, qh); V carries an
appended ones column so U row 64 is the softmax denominator. Denominators are
staged to SBUF during the loop; reciprocal (Ln+Exp), PE-matmul broadcast and
DVE normalize run batched at the tail. QKV matmuls not needed to start
attention are dribbled into the loop as PE filler. LayerNorm stats run in
bf16, with mean/bias folded into the QKV matmul via two appended rows.
"""

import numpy as np
import ml_dtypes

import concourse.bass as bass
import concourse.tile as tile
from concourse import bacc, mybir
from concourse.bass_utils import run_bass_kernel_spmd

F32 = mybir.dt.float32
BF16 = mybir.dt.bfloat16
AF = mybir.ActivationFunctionType
ALU = mybir.AluOpType

DIM = 768
HEADS = 12
B, N = 4, 2048
D = 64          # head dim
NQ = 1024       # queries per core
CT = 6          # 768 / 128 channel tiles
NT = 16         # 2048 / 128 token tiles
HP = 6          # head pairs

LAST = None  # BassKernelResults of the most recent run (for test harness)
_NC = None


def build():
    nc = bacc.Bacc("TRN2", target_bir_lowering=False, debug=False, num_devices=8)

    xT = nc.dram_tensor("xT", [DIM, N], BF16, kind="ExternalInput")
    wqkvT = nc.dram_tensor("wqkvT", [DIM + 2, 3 * DIM], BF16, kind="ExternalInput")
    woutT = nc.dram_tensor("woutT", [DIM + 1, DIM], BF16, kind="ExternalInput")
    selIn = nc.dram_tensor("selIn", [HEADS, HEADS * D], BF16, kind="ExternalInput")
    outT = nc.dram_tensor("outT", [DIM, NQ], F32, kind="ExternalOutput")
    import os
    dbg = os.environ.get("KDEBUG", "0") == "1"
    if dbg:
        d_xt = nc.dram_tensor("d_xt", [128, CT, N], BF16, kind="ExternalOutput")
        d_KT = nc.dram_tensor("d_KT", [128, CT, N], BF16, kind="ExternalOutput")
        d_QT = nc.dram_tensor("d_QT", [128, CT, NQ], BF16, kind="ExternalOutput")
        d_V4 = nc.dram_tensor("d_V4", [128, NT, HEADS, D + 1], BF16, kind="ExternalOutput")
        d_AO = nc.dram_tensor("d_AO", [128, HP, NQ], BF16, kind="ExternalOutput")
        d_den = nc.dram_tensor("d_den", [HEADS, NQ], BF16, kind="ExternalOutput")
        d_rb = nc.dram_tensor("d_rb", [HEADS, NQ], BF16, kind="ExternalOutput")

    with tile.TileContext(nc) as tc:
        with (
            tc.tile_pool(name="persist", bufs=1) as P1,
            tc.tile_pool(name="work", bufs=2) as PW,
            tc.tile_pool(name="et", bufs=3) as PET,
            tc.tile_pool(name="ps", bufs=1, space="PSUM") as PS,
        ):
            # ---- persistent SBUF tensors ----
            wq = P1.tile([128, CT, 3 * DIM], BF16)       # W'' rows 0..767
            wex = P1.tile([2, 3 * DIM], BF16)            # W'' rows 768..769
            WO = P1.tile([128, CT, DIM], BF16)           # w_out^T  (f-major tiles)
            wob = P1.tile([1, DIM], BF16)                # b_out row
            xt = P1.tile([128, CT, N], BF16, tag="big_a")  # bf16 x, then x~=x*rstd
            xex = P1.tile([2, N], BF16)                  # x~ rows 768 (-mu*rstd), 769 (1)
            KT = P1.tile([128, CT, N], BF16)             # K^T channel-major
            QT = P1.tile([128, CT, NQ], BF16)            # Q^T channel-major
            V4 = P1.tile([128, NT, HEADS, D + 1], BF16)  # V token-major + ones col
            AO = P1.tile([128, HP, NQ], BF16)            # attention out, f-major
            ones1b = P1.tile([128, 1], BF16)             # ones column (stats lhsT)
            sel = P1.tile([HEADS, HEADS, D], BF16)       # one-hot rows: sel[k,k,:]=1
            onesr = P1.tile([1, 128], BF16)              # ones row (warmup lhsT)
            onesrs = P1.tile([1, 128], BF16)             # sqrt(768) row (rstd bcast)
            onesI = P1.tile([1, NQ], BF16)               # ones row (bias rhs)
            epsc = P1.tile([1, 1], F32)
            lnDc = P1.tile([1, 1], F32)
            r_a = P1.tile([1, N], F32)                   # E[x^2] -> var -> rstd
            r_b = P1.tile([1, N], F32)                   # mu -> mu*rstd
            r_ab = P1.tile([1, N], BF16)                 # rstd bf16 (bcast rhs)
            rb_sb = P1.tile([128, 2, NQ], BF16)          # rstd bcast to 128 parts/half
            den_all = P1.tile([HEADS, NQ], BF16)          # softmax denominators
            den_st = P1.tile([65, NQ], BF16)              # staging (row 64 used)
            rb_all = P1.tile([HEADS, NQ], BF16)          # 1/den bf16

            nc.vector.memset(epsc[:], float(DIM) * 1e-5)
            nc.vector.memset(ones1b[:], 1.0)
            nc.vector.memset(onesr[:], 1.0)
            nc.vector.memset(onesrs[:], 1.0)
            nc.vector.memset(lnDc[:], 0.5 * float(np.log(DIM)))
            nc.vector.memset(onesI[:], 1.0)
            # row 1 must stay 1.0; row 0 is overwritten with -mu*rstd below
            nc.vector.memset(xex[:, :], 1.0)
            nc.vector.memset(V4[:, :, :, D : D + 1], 1.0)

            # ---- PE warmup spin + ACT table preload (runs during the x DMA;
            # keeps HAM at K=8/8 so head/early-D matmuls run at full clock) ----
            warm = PS.tile([128, 512], F32, tag="fill", bufs=2, name="warm")
            for i in range(26):
                nc.tensor.matmul(
                    warm[:], onesr[:], onesI[:, 0:512],
                    start=(i == 0), stop=(i == 25),
                )
            nc.scalar.activation(epsc[:], epsc[:], AF.Ln, bias=1.0)
            nc.vector.memset(epsc[:], float(DIM) * 1e-5)

            # ---- DMAs: x first (gates everything), weights on the gpsimd queue ----
            for h in range(2):
                for ct in range(CT):
                    nc.sync.dma_start(
                        xt[:, ct, h * 1024 : (h + 1) * 1024],
                        xT[ct * 128 : (ct + 1) * 128, h * 1024 : (h + 1) * 1024],
                    )
            for ct in range(CT):
                nc.gpsimd.dma_start(wq[:, ct, :], wqkvT[ct * 128 : (ct + 1) * 128, :])
            nc.gpsimd.dma_start(wex[:], wqkvT[DIM : DIM + 2, :])
            for ct in range(CT):
                nc.gpsimd.dma_start(WO[:, ct, :], woutT[ct * 128 : (ct + 1) * 128, :])
            nc.gpsimd.dma_start(wob[:], woutT[DIM : DIM + 1, :])
            nc.gpsimd.dma_start(sel[:].rearrange("h k d -> h (k d)"), selIn[:, :])

            # ---- phase A: per-half LN stats (bf16) + rstd + x~ ----
            for h in range(2):
                hsl = slice(h * 1024, (h + 1) * 1024)
                sts = PS.tile([128, 1024], F32, tag="sp", bufs=2, name=f"sts_{h}")
                for ct in range(CT):
                    xsq = PW.tile([128, 1024], BF16, tag="xsq", name=f"xsq_{h}_{ct}")
                    nc.vector.tensor_tensor(
                        xsq[:], xt[:, ct, hsl], xt[:, ct, hsl], ALU.mult
                    )
                    for s in range(2):
                        osl = slice(s * 512, (s + 1) * 512)
                        csl = slice(h * 1024 + s * 512, h * 1024 + (s + 1) * 512)
                        nc.tensor.matmul(
                            sts[0:1, osl], ones1b[:], xt[:, ct, csl],
                            start=(ct == 0), stop=(ct == CT - 1),
                        )
                        nc.tensor.matmul(
                            sts[64:65, osl], ones1b[:], xsq[:, osl],
                            start=(ct == 0), stop=(ct == CT - 1),
                        )
                # u = S1^2/768 (ACT), t = S2 - u (DVE), then
                # rstd/sqrt(768) = exp(-0.5*ln(t + 768*eps)); the sqrt(768)
                # factor is folded into the broadcast ones row (onesrs).
                nc.scalar.activation(
                    r_b[:, hsl], sts[0:1, :], AF.Square, scale=DIM ** -0.5
                )
                nc.vector.tensor_tensor(
                    r_a[:, hsl], sts[64:65, :], r_b[:, hsl], ALU.subtract
                )
                nc.scalar.activation(r_a[:, hsl], r_a[:, hsl], AF.Ln, bias=epsc[:])
                nc.scalar.activation(
                    r_a[:, hsl], r_a[:, hsl], AF.Exp, scale=-0.5, bias=lnDc[:]
                )
                nc.vector.tensor_copy(r_ab[:, hsl], r_a[:, hsl])    # bf16 for bcast MM
                # broadcast rstd over 128 partitions via PE, stage to SBUF bf16
                rb_ps = PS.tile([128, 1024], F32, tag="U", bufs=1, name=f"rb_ps_{h}")
                for s in range(2):
                    nc.tensor.matmul(
                        rb_ps[:, s * 512 : (s + 1) * 512],
                        onesrs[:],
                        r_ab[:, h * 1024 + s * 512 : h * 1024 + (s + 1) * 512],
                    )
                nc.vector.tensor_copy(rb_sb[:, h, :], rb_ps[:])
                # x~ = x * rstd in place (bf16 * bf16)
                for ct in range(CT):
                    nc.vector.tensor_tensor(
                        xt[:, ct, hsl], xt[:, ct, hsl], rb_sb[:, h, :], ALU.mult
                    )
                # folded-LN extra row: -mu*rstd = -(S1 * r_a)/sqrt(768)
                nc.vector.tensor_tensor(r_b[:, hsl], sts[0:1, :], r_a[:, hsl], ALU.mult)
                nc.vector.tensor_scalar_mul(xex[0:1, hsl], r_b[:, hsl], -1.0 / DIM)

            # ---- filler work queue ----
            def kq_unit(which, fidx, nh, s, scalar_copy=False):
                base = DIM if which == "K" else 0
                fsl = slice(base + fidx * 128, base + (fidx + 1) * 128)
                n0 = nh * 1024 + s * 512
                dst = (KT if which == "K" else QT)[:, fidx, n0 : n0 + 512]

                def go():
                    acc = PS.tile(
                        [128, 512], F32, tag="fill", bufs=2,
                        name=f"kq{which}_{fidx}_{nh}_{s}",
                    )
                    for ct in range(CT + 1):
                        yield nc.tensor.matmul(
                            acc[:],
                            wq[:, ct, fsl] if ct < CT else wex[:, fsl],
                            xt[:, ct, n0 : n0 + 512] if ct < CT else xex[:, n0 : n0 + 512],
                            start=(ct == 0), stop=(ct == CT),
                        )
                    if scalar_copy:
                        nc.scalar.copy(dst, acc[:])
                    else:
                        nc.vector.tensor_copy(dst, acc[:])
                return go

            def v_unit(nt, lo, sz, scalar_copy=False):
                nsl = slice(nt * 128, (nt + 1) * 128)
                h0 = lo // 64  # first head covered

                def go():
                    acc = PS.tile(
                        [128, 512], F32, tag="fill", bufs=2, name=f"v_{nt}_{lo}"
                    )
                    for ct in range(CT + 1):
                        fsl = slice(2 * DIM + lo, 2 * DIM + lo + sz)
                        yield nc.tensor.matmul(
                            acc[:, 0:sz],
                            xt[:, ct, nsl] if ct < CT else xex[:, nsl],
                            wq[:, ct, fsl] if ct < CT else wex[:, fsl],
                            start=(ct == 0), stop=(ct == CT),
                        )
                    cp = nc.scalar.copy if scalar_copy else nc.vector.tensor_copy
                    cp(
                        V4[:, nt, h0 : h0 + sz // 64, 0:D],
                        acc[:, 0:sz].rearrange("p (h d) -> p h d", d=D),
                    )
                return go

            class Filler:
                """Work queue of matmul-chain generators, each with a deadline
                (iteration index). emit(i) advances the queue by the normal
                quota but ALWAYS finishes every unit whose deadline is <= i:
                a unit's instructions must be emitted in program order before
                the attention instruction that reads its output, or the
                dependency is silently missed (read-before-write)."""

                def __init__(self):
                    self.units = []   # (deadline, generator-fn)
                    self.cur = None
                    self.cur_deadline = None

                def add(self, deadline, go):
                    self.units.append((deadline, go))

                def emit(self, i, quota):
                    while True:
                        if self.cur is None:
                            if not self.units:
                                return
                            if quota <= 0 and self.units[0][0] > i:
                                return
                            self.cur_deadline, go = self.units.pop(0)
                            self.cur = go()
                        if quota <= 0 and self.cur_deadline > i:
                            return
                        try:
                            next(self.cur)
                            quota -= 1
                        except StopIteration:
                            self.cur = None

                def drain(self):
                    self.emit(10 ** 9, 10 ** 9)

            fill = Filler()

            # head: K/Q pair 0 over the local queries (jt 0-3) + V tile 0
            for go in (
                [kq_unit("K", 0, 0, s) for s in range(2)]
                + [kq_unit("Q", 0, 0, s) for s in range(2)]
                + [v_unit(nt, 0, 512) for nt in range(10)]
            ):
                for _ in go():
                    pass

            # filler with deadlines (iteration index in the 192-iter space):
            #   V-512 tile j feeds AV at iter j (and 16+j); K(p,nh,s) feeds
            #   scores at iter 32p + nh*8 + s*4 (issued one iter early);
            #   Q(p,s) feeds qh=s pass of pair p; V-256 tile j feeds hp4.
            fill.add(6, kq_unit("K", 0, 1, 0))
            fill.add(10, kq_unit("K", 0, 1, 1))

            for nt in range(10, NT):
                fill.add(max(nt - 2, 0), v_unit(nt, 0, 512))
            for p in range(1, HP):
                d = 32 * p - 2
                fill.add(d, kq_unit("K", p, 0, 0))
                fill.add(d, kq_unit("Q", p, 0, 0))
                fill.add(d + 2, kq_unit("K", p, 0, 1))
                fill.add(d + 8, kq_unit("K", p, 1, 0))
                fill.add(d + 10, kq_unit("K", p, 1, 1))
                fill.add(d + 14, kq_unit("Q", p, 0, 1))
                if p == 2:
                    for nt in range(NT):
                        fill.add(64 + 4 * nt, v_unit(nt, 512, 256))

            # ---- phase D: attention, software-pipelined ----
            scale = float(D) ** -0.5
            iters = [
                (hp, qh, jt) for hp in range(HP) for qh in range(2) for jt in range(NT)
            ]
            sp_tiles = {}

            def issue_scores(hp, qh, jt):
                sp = PS.tile(
                    [128, 1024], F32, tag="sp", bufs=2, name=f"sp_{hp}_{qh}_{jt}"
                )
                sp_tiles[(hp, qh, jt)] = sp
                jsl = slice(jt * 128, (jt + 1) * 128)
                qsl = slice(qh * 512, (qh + 1) * 512)
                # h0 rows 0:64, h1 rows 64:128 -> adjacent MMs stream concurrently
                nc.tensor.matmul(
                    sp[:, 0:512], KT[0:64, hp, jsl], QT[0:64, hp, qsl],
                    start=True, stop=True,
                )
                nc.tensor.matmul(
                    sp[:, 512:1024], KT[64:128, hp, jsl], QT[64:128, hp, qsl],
                    start=True, stop=True,
                )

            fill.units.sort(key=lambda u: u[0])
            issue_scores(*iters[0])
            U = None
            for idx, (hp, qh, jt) in enumerate(iters):
                if idx + 1 < len(iters):
                    issue_scores(*iters[idx + 1])
                sp = sp_tiles.pop((hp, qh, jt))
                ET = PET.tile([128, 1024], BF16, tag="et", name=f"et_{hp}_{qh}_{jt}")
                nc.scalar.activation(ET[:], sp[:], AF.Exp, scale=scale)
                if jt == 0:
                    U = PS.tile([128, 1024], F32, tag="U", bufs=1, name=f"U_{hp}_{qh}")
                nc.tensor.matmul(
                    U[0 : D + 1, 0:512], V4[:, jt, 2 * hp, :], ET[:, 0:512],
                    start=(jt == 0), stop=(jt == NT - 1),
                )
                nc.tensor.matmul(
                    U[0 : D + 1, 512:1024], V4[:, jt, 2 * hp + 1, :], ET[:, 512:1024],
                    start=(jt == 0), stop=(jt == NT - 1),
                )
                fill.emit(idx, 2)
                if jt == NT - 1:
                    # drain U: raw AV to AO (bf16), denominators to SBUF staging
                    qsl = slice(qh * 512, (qh + 1) * 512)
                    nc.vector.tensor_copy(AO[0:64, hp, qsl], U[0:64, 0:512])
                    AOtmp = PW.tile(
                        [64, 512], BF16, tag="AOtmp", name=f"AOtmp_{hp}_{qh}"
                    )
                    nc.vector.tensor_copy(AOtmp[:], U[0:64, 512:1024])
                    nc.sync.dma_start(AO[64:128, hp, qsl], AOtmp[:])
                    nc.vector.tensor_copy(den_st[64:65, :], U[64:65, :])
                    nc.sync.dma_start(
                        den_all[2 * hp : 2 * hp + 1, qsl], den_st[64:65, 0:512]
                    )
                    nc.sync.dma_start(
                        den_all[2 * hp + 1 : 2 * hp + 2, qsl], den_st[64:65, 512:1024]
                    )
            fill.drain()

            # ---- tail: reciprocal, broadcast, normalize, out-projection ----
            # out-proj bias matmuls first: dependency-free PE work that keeps
            # the HAM warm while the reciprocal chain (2 table loads + Ln+Exp)
            # runs on ACT.
            po_tiles = {}
            for ot in range(2):
                osl = slice(ot * 128, (ot + 1) * 128)
                po = PS.tile([128, 1024], F32, tag="sp", bufs=2, name=f"po_{ot}")
                po_tiles[ot] = po
                for s in range(2):
                    ssl = slice(s * 512, (s + 1) * 512)
                    nc.tensor.matmul(
                        po[:, ssl], wob[:, osl], onesI[:, ssl],
                        start=True, stop=False,
                    )
            lnd_ps = PS.tile([128, 1024], F32, tag="U", bufs=1, name="lnd_ps")
            nc.scalar.activation(lnd_ps[0:HEADS, :], den_all[:], AF.Ln)
            nc.scalar.activation(rb_all[:], lnd_ps[0:HEADS, :], AF.Exp, scale=-1.0)
            for hp in range(HP):
                # broadcast 1/den rows over 64 partitions via one-hot selector:
                # rbB[0:64] = sel[:,2hp,:].T @ rb_all[0:12], rbB[64:128] likewise
                rbB = PS.tile([128, 1024], F32, tag="U", bufs=1, name=f"rbB_{hp}")
                for s in range(2):
                    ssl = slice(s * 512, (s + 1) * 512)
                    nc.tensor.matmul(
                        rbB[0:64, ssl], sel[:, 2 * hp, :], rb_all[:, ssl],
                    )
                    nc.tensor.matmul(
                        rbB[64:128, ssl], sel[:, 2 * hp + 1, :], rb_all[:, ssl],
                    )
                nc.vector.tensor_tensor(AO[:, hp, :], AO[:, hp, :], rbB[:], ALU.mult)

            if dbg:
                nc.sync.dma_start(d_xt[:], xt[:])
                nc.sync.dma_start(d_KT[:], KT[:])
                nc.sync.dma_start(d_QT[:], QT[:])
                nc.sync.dma_start(d_V4[:], V4[:])
                nc.sync.dma_start(d_AO[:], AO[:])
                nc.sync.dma_start(d_den[:], den_all[:])
                nc.sync.dma_start(d_rb[:], rb_all[:])

            for ot in range(CT):
                osl = slice(ot * 128, (ot + 1) * 128)
                if ot in po_tiles:
                    po = po_tiles[ot]
                else:
                    po = PS.tile([128, 1024], F32, tag="sp", bufs=2, name=f"po_{ot}")
                for s in range(2):
                    ssl = slice(s * 512, (s + 1) * 512)
                    if ot not in po_tiles:
                        nc.tensor.matmul(
                            po[:, ssl], wob[:, osl], onesI[:, ssl],
                            start=True, stop=False,
                        )
                    for ft in range(CT):
                        nc.tensor.matmul(
                            po[:, ssl], WO[:, ft, osl], AO[:, ft, ssl],
                            start=False, stop=(ft == CT - 1),
                        )
                outsb = PW.tile([128, 1024], F32, tag="outsb", bufs=1, name=f"outsb_{ot}")
                nc.scalar.copy(outsb[:], po[:])
                nc.sync.dma_start(outT[osl, :], outsb[:])

    nc.finalize()
    return nc


def _get_nc():
    global _NC
    if _NC is None:
        _NC = build()
    return _NC


def kernel(x, ln_w, ln_b, w_qkv, w_out, b_out):
    global LAST
    x = np.asarray(x, dtype=np.float32)
    ln_w = np.asarray(ln_w, dtype=np.float32)
    ln_b = np.asarray(ln_b, dtype=np.float32)
    w_qkv = np.asarray(w_qkv, dtype=np.float32)
    w_out = np.asarray(w_out, dtype=np.float32)
    b_out = np.asarray(b_out, dtype=np.float32)

    bf16 = ml_dtypes.bfloat16
    # W'' = [ (w_qkv * ln_w)^T ; rowsum of (w_qkv*ln_w) ; w_qkv @ ln_b ]
    wprime = w_qkv * ln_w[None, :]
    wqkvT = np.concatenate(
        [wprime.T, wprime.sum(axis=1)[None, :], (w_qkv @ ln_b)[None, :]], axis=0
    ).astype(bf16)
    woutT = np.concatenate([w_out.T, b_out[None, :]], axis=0).astype(bf16)
    selmat = np.kron(np.eye(HEADS, dtype=np.float32), np.ones((1, D), np.float32)).astype(bf16)

    in_maps = []
    for c in range(8):
        b, g = c // 2, c % 2
        order = np.r_[g * NQ : (g + 1) * NQ, (1 - g) * NQ : (2 - g) * NQ]
        xTc = np.ascontiguousarray(x[b][order].T).astype(bf16)
        in_maps.append({"xT": xTc, "wqkvT": wqkvT, "woutT": woutT, "selIn": selmat})

    nc = _get_nc()
    LAST = run_bass_kernel_spmd(nc, in_maps, core_ids=list(range(8)))

    out = np.empty((B, N, DIM), dtype=np.float32)
    for c in range(8):
        b, g = c // 2, c % 2
        out[b, g * NQ : (g + 1) * NQ, :] = LAST.results[c]["outT"].T
    return out


# revision 31
# speedup vs baseline: 1.0221x; 1.0175x over previous
"""Fused LayerNorm + multi-head attention Trainium2 kernel, 8-core SPMD.

Problem: x[4, 2048, 768] -> LN -> QKV (w_qkv[2304, 768]) -> 12-head attention
         -> out proj (w_out[768, 768] + b_out). f32 I/O, bf16 tensor-engine compute.

Sharding: core c handles batch b=c//2, query-half g=c%2 (1024 queries each).
Each core receives the FULL (rotated) sequence of its batch so K/V are computed
locally -- no collectives. The token order is rotated per-core so the core's own
query chunk is always columns [0, 1024) => identical SPMD program on all cores.

v2: the attention loop is software-pipelined so the ACT engine (exp) never
starves. Per (head-pair hp, query-half qh, kv-tile jt): one [128,1024] PSUM
scores tile holds both heads of the pair (h0 cols 0:512, h1 cols 512:1024),
one exp call covers both, and the two scores matmuls use disjoint PE row
groups (partitions 0:64 vs 64:128) so they stream concurrently. AV
accumulates into a single U [128,1024] PSUM tile per (hp, qh); V carries an
appended ones column so U row 64 is the softmax denominator. Denominators are
staged to SBUF during the loop; reciprocal (Ln+Exp), PE-matmul broadcast and
DVE normalize run batched at the tail. QKV matmuls not needed to start
attention are dribbled into the loop as PE filler. LayerNorm stats run in
bf16, with mean/bias folded into the QKV matmul via two appended rows.
"""

import numpy as np
import ml_dtypes

import concourse.bass as bass
import concourse.tile as tile
from concourse import bacc, mybir
from concourse.bass_utils import run_bass_kernel_spmd

F32 = mybir.dt.float32
BF16 = mybir.dt.bfloat16
AF = mybir.ActivationFunctionType
ALU = mybir.AluOpType

DIM = 768
HEADS = 12
B, N = 4, 2048
D = 64          # head dim
NQ = 1024       # queries per core
CT = 6          # 768 / 128 channel tiles
NT = 16         # 2048 / 128 token tiles
HP = 6          # head pairs

LAST = None  # BassKernelResults of the most recent run (for test harness)
_NC = None


def build():
    nc = bacc.Bacc("TRN2", target_bir_lowering=False, debug=False, num_devices=8)

    xT = nc.dram_tensor("xT", [DIM, N], BF16, kind="ExternalInput")
    wqkvT = nc.dram_tensor("wqkvT", [DIM + 2, 3 * DIM], BF16, kind="ExternalInput")
    woutT = nc.dram_tensor("woutT", [DIM + 1, DIM], BF16, kind="ExternalInput")
    selIn = nc.dram_tensor("selIn", [HEADS, HEADS * D], BF16, kind="ExternalInput")
    outT = nc.dram_tensor("outT", [DIM, NQ], F32, kind="ExternalOutput")
    import os
    dbg = os.environ.get("KDEBUG", "0") == "1"
    if dbg:
        d_xt = nc.dram_tensor("d_xt", [128, CT, N], BF16, kind="ExternalOutput")
        d_KT = nc.dram_tensor("d_KT", [128, CT, N], BF16, kind="ExternalOutput")
        d_QT = nc.dram_tensor("d_QT", [128, CT, NQ], BF16, kind="ExternalOutput")
        d_V4 = nc.dram_tensor("d_V4", [128, NT, HEADS, D + 1], BF16, kind="ExternalOutput")
        d_AO = nc.dram_tensor("d_AO", [128, HP, NQ], BF16, kind="ExternalOutput")
        d_den = nc.dram_tensor("d_den", [HEADS, NQ], BF16, kind="ExternalOutput")
        d_rb = nc.dram_tensor("d_rb", [HEADS, NQ], BF16, kind="ExternalOutput")

    with tile.TileContext(nc) as tc:
        with (
            tc.tile_pool(name="persist", bufs=1) as P1,
            tc.tile_pool(name="work", bufs=2) as PW,
            tc.tile_pool(name="et", bufs=3) as PET,
            tc.tile_pool(name="ps", bufs=1, space="PSUM") as PS,
        ):
            # ---- persistent SBUF tensors ----
            wq = P1.tile([128, CT, 3 * DIM], BF16)       # W'' rows 0..767
            wex = P1.tile([2, 3 * DIM], BF16)            # W'' rows 768..769
            WO = P1.tile([128, CT, DIM], BF16)           # w_out^T  (f-major tiles)
            wob = P1.tile([1, DIM], BF16)                # b_out row
            xt = P1.tile([128, CT, N], BF16, tag="big_a")  # bf16 x, then x~=x*rstd
            xex = P1.tile([2, N], BF16)                  # x~ rows 768 (-mu*rstd), 769 (1)
            KT = P1.tile([128, CT, N], BF16)             # K^T channel-major
            QT = P1.tile([128, CT, NQ], BF16)            # Q^T channel-major
            V4 = P1.tile([128, NT, HEADS, D + 1], BF16)  # V token-major + ones col
            AO = P1.tile([128, HP, NQ], BF16)            # attention out, f-major
            ones1b = P1.tile([128, 1], BF16)             # ones column (stats lhsT)
            sel = P1.tile([HEADS, HEADS, D], BF16)       # one-hot rows: sel[k,k,:]=1
            onesr = P1.tile([1, 128], BF16)              # ones row (warmup lhsT)
            onesrs = P1.tile([1, 128], BF16)             # sqrt(768) row (rstd bcast)
            onesI = P1.tile([1, NQ], BF16)               # ones row (bias rhs)
            epsc = P1.tile([1, 1], F32)
            lnDc = P1.tile([1, 1], F32)
            r_a = P1.tile([1, N], F32)                   # E[x^2] -> var -> rstd
            r_b = P1.tile([1, N], F32)                   # mu -> mu*rstd
            r_ab = P1.tile([1, N], BF16)                 # rstd bf16 (bcast rhs)
            rb_sb = P1.tile([128, 2, NQ], BF16)          # rstd bcast to 128 parts/half
            den_all = P1.tile([HEADS, NQ], BF16)          # softmax denominators
            den_st = P1.tile([65, NQ], BF16)              # staging (row 64 used)
            rb_all = P1.tile([HEADS, NQ], BF16)          # 1/den bf16

            nc.vector.memset(epsc[:], float(DIM) * 1e-5)
            nc.vector.memset(ones1b[:], 1.0)
            nc.vector.memset(onesr[:], 1.0)
            nc.vector.memset(onesrs[:], 1.0)
            nc.vector.memset(lnDc[:], 0.5 * float(np.log(DIM)))
            nc.vector.memset(onesI[:], 1.0)
            # row 1 must stay 1.0; row 0 is overwritten with -mu*rstd below
            nc.vector.memset(xex[:, :], 1.0)
            nc.vector.memset(V4[:, :, :, D : D + 1], 1.0)

            # ---- PE warmup spin + ACT table preload (runs during the x DMA;
            # keeps HAM at K=8/8 so head/early-D matmuls run at full clock) ----
            warm = PS.tile([128, 512], F32, tag="fill", bufs=2, name="warm")
            for i in range(26):
                nc.tensor.matmul(
                    warm[:], onesr[:], onesI[:, 0:512],
                    start=(i == 0), stop=(i == 25),
                )
            nc.scalar.activation(epsc[:], epsc[:], AF.Ln, bias=1.0)
            nc.vector.memset(epsc[:], float(DIM) * 1e-5)

            # ---- DMAs: x first (gates everything), weights on the gpsimd queue ----
            for h in range(2):
                for ct in range(CT):
                    nc.sync.dma_start(
                        xt[:, ct, h * 1024 : (h + 1) * 1024],
                        xT[ct * 128 : (ct + 1) * 128, h * 1024 : (h + 1) * 1024],
                    )
            for ct in range(CT):
                nc.gpsimd.dma_start(wq[:, ct, :], wqkvT[ct * 128 : (ct + 1) * 128, :])
            nc.gpsimd.dma_start(wex[:], wqkvT[DIM : DIM + 2, :])
            for ct in range(CT):
                nc.gpsimd.dma_start(WO[:, ct, :], woutT[ct * 128 : (ct + 1) * 128, :])
            nc.gpsimd.dma_start(wob[:], woutT[DIM : DIM + 1, :])
            nc.gpsimd.dma_start(sel[:].rearrange("h k d -> h (k d)"), selIn[:, :])

            # ---- phase A: per-half LN stats (bf16) + rstd + x~ ----
            # Split into stats (fast DVE ops) and chain (ACT-gated slow ops)
            # so head-unit copies can be emitted between them: the DVE queue
            # is in-order, and a slow not-ready op at its head blocks every
            # later (ready) copy.
            sts_t = {}

            def phase_a_stats(h):
                hsl = slice(h * 1024, (h + 1) * 1024)
                sts = PS.tile([128, 1024], F32, tag="sp", bufs=2, name=f"sts_{h}")
                sts_t[h] = sts
                for ct in range(CT):
                    xsq = PW.tile([128, 1024], BF16, tag="xsq", name=f"xsq_{h}_{ct}")
                    nc.vector.tensor_tensor(
                        xsq[:], xt[:, ct, hsl], xt[:, ct, hsl], ALU.mult
                    )
                    for s in range(2):
                        osl = slice(s * 512, (s + 1) * 512)
                        csl = slice(h * 1024 + s * 512, h * 1024 + (s + 1) * 512)
                        nc.tensor.matmul(
                            sts[0:1, osl], ones1b[:], xt[:, ct, csl],
                            start=(ct == 0), stop=(ct == CT - 1),
                        )
                        nc.tensor.matmul(
                            sts[64:65, osl], ones1b[:], xsq[:, osl],
                            start=(ct == 0), stop=(ct == CT - 1),
                        )

            def phase_a_chain(h):
                hsl = slice(h * 1024, (h + 1) * 1024)
                sts = sts_t[h]
                # u = S1^2/768 (ACT), t = S2 - u (DVE), then
                # rstd/sqrt(768) = exp(-0.5*ln(t + 768*eps)); the sqrt(768)
                # factor is folded into the broadcast ones row (onesrs).
                nc.scalar.activation(
                    r_b[:, hsl], sts[0:1, :], AF.Square, scale=DIM ** -0.5
                )
                nc.vector.tensor_tensor(
                    r_a[:, hsl], sts[64:65, :], r_b[:, hsl], ALU.subtract
                )
                nc.scalar.activation(r_a[:, hsl], r_a[:, hsl], AF.Ln, bias=epsc[:])
                nc.scalar.activation(
                    r_a[:, hsl], r_a[:, hsl], AF.Exp, scale=-0.5, bias=lnDc[:]
                )
                nc.vector.tensor_copy(r_ab[:, hsl], r_a[:, hsl])    # bf16 for bcast MM
                # broadcast rstd over 128 partitions via PE, stage to SBUF bf16
                rb_ps = PS.tile([128, 1024], F32, tag="U", bufs=1, name=f"rb_ps_{h}")
                for s in range(2):
                    nc.tensor.matmul(
                        rb_ps[:, s * 512 : (s + 1) * 512],
                        onesrs[:],
                        r_ab[:, h * 1024 + s * 512 : h * 1024 + (s + 1) * 512],
                    )
                nc.vector.tensor_copy(rb_sb[:, h, :], rb_ps[:])
                # x~ = x * rstd in place (bf16 * bf16)
                for ct in range(CT):
                    nc.vector.tensor_tensor(
                        xt[:, ct, hsl], xt[:, ct, hsl], rb_sb[:, h, :], ALU.mult
                    )
                # folded-LN extra row: -mu*rstd = -(S1 * r_a)/sqrt(768)
                nc.vector.tensor_tensor(r_b[:, hsl], sts[0:1, :], r_a[:, hsl], ALU.mult)
                nc.vector.tensor_scalar_mul(xex[0:1, hsl], r_b[:, hsl], -1.0 / DIM)

            # ---- filler work queue ----
            def kq_unit(which, fidx, nh, s, scalar_copy=False):
                base = DIM if which == "K" else 0
                fsl = slice(base + fidx * 128, base + (fidx + 1) * 128)
                n0 = nh * 1024 + s * 512
                dst = (KT if which == "K" else QT)[:, fidx, n0 : n0 + 512]

                def go():
                    acc = PS.tile(
                        [128, 512], F32, tag="fill", bufs=2,
                        name=f"kq{which}_{fidx}_{nh}_{s}",
                    )
                    for ct in range(CT + 1):
                        yield nc.tensor.matmul(
                            acc[:],
                            wq[:, ct, fsl] if ct < CT else wex[:, fsl],
                            xt[:, ct, n0 : n0 + 512] if ct < CT else xex[:, n0 : n0 + 512],
                            start=(ct == 0), stop=(ct == CT),
                        )
                    if scalar_copy:
                        nc.scalar.copy(dst, acc[:])
                    else:
                        nc.vector.tensor_copy(dst, acc[:])
                return go

            def v_unit(nt, lo, sz, scalar_copy=False):
                nsl = slice(nt * 128, (nt + 1) * 128)
                h0 = lo // 64  # first head covered

                def go():
                    acc = PS.tile(
                        [128, 512], F32, tag="fill", bufs=2, name=f"v_{nt}_{lo}"
                    )
                    for ct in range(CT + 1):
                        fsl = slice(2 * DIM + lo, 2 * DIM + lo + sz)
                        yield nc.tensor.matmul(
                            acc[:, 0:sz],
                            xt[:, ct, nsl] if ct < CT else xex[:, nsl],
                            wq[:, ct, fsl] if ct < CT else wex[:, fsl],
                            start=(ct == 0), stop=(ct == CT),
                        )
                    cp = nc.scalar.copy if scalar_copy else nc.vector.tensor_copy
                    cp(
                        V4[:, nt, h0 : h0 + sz // 64, 0:D],
                        acc[:, 0:sz].rearrange("p (h d) -> p h d", d=D),
                    )
                return go

            class Filler:
                """Work queue of matmul-chain generators, each with a deadline
                (iteration index). emit(i) advances the queue by the normal
                quota but ALWAYS finishes every unit whose deadline is <= i:
                a unit's instructions must be emitted in program order before
                the attention instruction that reads its output, or the
                dependency is silently missed (read-before-write)."""

                def __init__(self):
                    self.units = []   # (deadline, generator-fn)
                    self.cur = None
                    self.cur_deadline = None

                def add(self, deadline, go):
                    self.units.append((deadline, go))

                def emit(self, i, quota):
                    while True:
                        if self.cur is None:
                            if not self.units:
                                return
                            if quota <= 0 and self.units[0][0] > i:
                                return
                            self.cur_deadline, go = self.units.pop(0)
                            self.cur = go()
                        if quota <= 0 and self.cur_deadline > i:
                            return
                        try:
                            next(self.cur)
                            quota -= 1
                        except StopIteration:
                            self.cur = None

                def drain(self):
                    self.emit(10 ** 9, 10 ** 9)

            fill = Filler()

            # interleaved head: h0 stats+chain, K/Q pair 0 (h0), h1 stats,
            # V tiles 0-7 (h0 tokens), h1 chain, V tiles 8-9 (h1 tokens)
            phase_a_stats(0)
            phase_a_chain(0)
            for go in (
                [kq_unit("K", 0, 0, s) for s in range(2)]
                + [kq_unit("Q", 0, 0, s) for s in range(2)]
            ):
                for _ in go():
                    pass
            phase_a_stats(1)
            for go in [v_unit(nt, 0, 512) for nt in range(8)]:
                for _ in go():
                    pass
            phase_a_chain(1)
            for go in [v_unit(nt, 0, 512) for nt in range(8, 10)]:
                for _ in go():
                    pass

            # filler with deadlines (iteration index in the 192-iter space):
            #   V-512 tile j feeds AV at iter j (and 16+j); K(p,nh,s) feeds
            #   scores at iter 32p + nh*8 + s*4 (issued one iter early);
            #   Q(p,s) feeds qh=s pass of pair p; V-256 tile j feeds hp4.
            fill.add(6, kq_unit("K", 0, 1, 0))
            fill.add(10, kq_unit("K", 0, 1, 1))

            for nt in range(10, NT):
                fill.add(max(nt - 2, 0), v_unit(nt, 0, 512))
            for p in range(1, HP):
                d = 32 * p - 2
                fill.add(d, kq_unit("K", p, 0, 0))
                fill.add(d, kq_unit("Q", p, 0, 0))
                fill.add(d + 2, kq_unit("K", p, 0, 1))
                fill.add(d + 8, kq_unit("K", p, 1, 0))
                fill.add(d + 10, kq_unit("K", p, 1, 1))
                fill.add(d + 14, kq_unit("Q", p, 0, 1))
                if p == 2:
                    for nt in range(NT):
                        fill.add(64 + 4 * nt, v_unit(nt, 512, 256))

            # ---- phase D: attention, software-pipelined ----
            scale = float(D) ** -0.5
            iters = [
                (hp, qh, jt) for hp in range(HP) for qh in range(2) for jt in range(NT)
            ]
            sp_tiles = {}

            def issue_scores(hp, qh, jt):
                sp = PS.tile(
                    [128, 1024], F32, tag="sp", bufs=2, name=f"sp_{hp}_{qh}_{jt}"
                )
                sp_tiles[(hp, qh, jt)] = sp
                jsl = slice(jt * 128, (jt + 1) * 128)
                qsl = slice(qh * 512, (qh + 1) * 512)
                # h0 rows 0:64, h1 rows 64:128 -> adjacent MMs stream concurrently
                nc.tensor.matmul(
                    sp[:, 0:512], KT[0:64, hp, jsl], QT[0:64, hp, qsl],
                    start=True, stop=True,
                )
                nc.tensor.matmul(
                    sp[:, 512:1024], KT[64:128, hp, jsl], QT[64:128, hp, qsl],
                    start=True, stop=True,
                )

            fill.units.sort(key=lambda u: u[0])
            issue_scores(*iters[0])
            U = None
            for idx, (hp, qh, jt) in enumerate(iters):
                if idx + 1 < len(iters):
                    issue_scores(*iters[idx + 1])
                sp = sp_tiles.pop((hp, qh, jt))
                ET = PET.tile([128, 1024], BF16, tag="et", name=f"et_{hp}_{qh}_{jt}")
                nc.scalar.activation(ET[:], sp[:], AF.Exp, scale=scale)
                if jt == 0:
                    U = PS.tile([128, 1024], F32, tag="U", bufs=1, name=f"U_{hp}_{qh}")
                nc.tensor.matmul(
                    U[0 : D + 1, 0:512], V4[:, jt, 2 * hp, :], ET[:, 0:512],
                    start=(jt == 0), stop=(jt == NT - 1),
                )
                nc.tensor.matmul(
                    U[0 : D + 1, 512:1024], V4[:, jt, 2 * hp + 1, :], ET[:, 512:1024],
                    start=(jt == 0), stop=(jt == NT - 1),
                )
                fill.emit(idx, 2)
                if jt == NT - 1:
                    # drain U: raw AV to AO (bf16), denominators to SBUF staging
                    qsl = slice(qh * 512, (qh + 1) * 512)
                    nc.vector.tensor_copy(AO[0:64, hp, qsl], U[0:64, 0:512])
                    AOtmp = PW.tile(
                        [64, 512], BF16, tag="AOtmp", name=f"AOtmp_{hp}_{qh}"
                    )
                    nc.vector.tensor_copy(AOtmp[:], U[0:64, 512:1024])
                    nc.sync.dma_start(AO[64:128, hp, qsl], AOtmp[:])
                    nc.vector.tensor_copy(den_st[64:65, :], U[64:65, :])
                    nc.sync.dma_start(
                        den_all[2 * hp : 2 * hp + 1, qsl], den_st[64:65, 0:512]
                    )
                    nc.sync.dma_start(
                        den_all[2 * hp + 1 : 2 * hp + 2, qsl], den_st[64:65, 512:1024]
                    )
            fill.drain()

            # ---- tail: reciprocal, broadcast, normalize, out-projection ----
            # out-proj bias matmuls first: dependency-free PE work that keeps
            # the HAM warm while the reciprocal chain (2 table loads + Ln+Exp)
            # runs on ACT.
            po_tiles = {}
            for ot in range(2):
                osl = slice(ot * 128, (ot + 1) * 128)
                po = PS.tile([128, 1024], F32, tag="sp", bufs=2, name=f"po_{ot}")
                po_tiles[ot] = po
                for s in range(2):
                    ssl = slice(s * 512, (s + 1) * 512)
                    nc.tensor.matmul(
                        po[:, ssl], wob[:, osl], onesI[:, ssl],
                        start=True, stop=False,
                    )
            lnd_ps = PS.tile([128, 1024], F32, tag="U", bufs=1, name="lnd_ps")
            nc.scalar.activation(lnd_ps[0:HEADS, :], den_all[:], AF.Ln)
            nc.scalar.activation(rb_all[:], lnd_ps[0:HEADS, :], AF.Exp, scale=-1.0)
            for hp in range(HP):
                # broadcast 1/den rows over 64 partitions via one-hot selector:
                # rbB[0:64] = sel[:,2hp,:].T @ rb_all[0:12], rbB[64:128] likewise
                rbB = PS.tile([128, 1024], F32, tag="U", bufs=1, name=f"rbB_{hp}")
                for s in range(2):
                    ssl = slice(s * 512, (s + 1) * 512)
                    nc.tensor.matmul(
                        rbB[0:64, ssl], sel[:, 2 * hp, :], rb_all[:, ssl],
                    )
                    nc.tensor.matmul(
                        rbB[64:128, ssl], sel[:, 2 * hp + 1, :], rb_all[:, ssl],
                    )
                nc.vector.tensor_tensor(AO[:, hp, :], AO[:, hp, :], rbB[:], ALU.mult)

            if dbg:
                nc.sync.dma_start(d_xt[:], xt[:])
                nc.sync.dma_start(d_KT[:], KT[:])
                nc.sync.dma_start(d_QT[:], QT[:])
                nc.sync.dma_start(d_V4[:], V4[:])
                nc.sync.dma_start(d_AO[:], AO[:])
                nc.sync.dma_start(d_den[:], den_all[:])
                nc.sync.dma_start(d_rb[:], rb_all[:])

            for ot in range(CT):
                osl = slice(ot * 128, (ot + 1) * 128)
                if ot in po_tiles:
                    po = po_tiles[ot]
                else:
                    po = PS.tile([128, 1024], F32, tag="sp", bufs=2, name=f"po_{ot}")
                for s in range(2):
                    ssl = slice(s * 512, (s + 1) * 512)
                    if ot not in po_tiles:
                        nc.tensor.matmul(
                            po[:, ssl], wob[:, osl], onesI[:, ssl],
                            start=True, stop=False,
                        )
                    for ft in range(CT):
                        nc.tensor.matmul(
                            po[:, ssl], WO[:, ft, osl], AO[:, ft, ssl],
                            start=False, stop=(ft == CT - 1),
                        )
                outsb = PW.tile([128, 1024], F32, tag="outsb", bufs=1, name=f"outsb_{ot}")
                nc.scalar.copy(outsb[:], po[:])
                nc.sync.dma_start(outT[osl, :], outsb[:])

    nc.finalize()
    return nc


def _get_nc():
    global _NC
    if _NC is None:
        _NC = build()
    return _NC


def kernel(x, ln_w, ln_b, w_qkv, w_out, b_out):
    global LAST
    x = np.asarray(x, dtype=np.float32)
    ln_w = np.asarray(ln_w, dtype=np.float32)
    ln_b = np.asarray(ln_b, dtype=np.float32)
    w_qkv = np.asarray(w_qkv, dtype=np.float32)
    w_out = np.asarray(w_out, dtype=np.float32)
    b_out = np.asarray(b_out, dtype=np.float32)

    bf16 = ml_dtypes.bfloat16
    # W'' = [ (w_qkv * ln_w)^T ; rowsum of (w_qkv*ln_w) ; w_qkv @ ln_b ]
    wprime = w_qkv * ln_w[None, :]
    wqkvT = np.concatenate(
        [wprime.T, wprime.sum(axis=1)[None, :], (w_qkv @ ln_b)[None, :]], axis=0
    ).astype(bf16)
    woutT = np.concatenate([w_out.T, b_out[None, :]], axis=0).astype(bf16)
    selmat = np.kron(np.eye(HEADS, dtype=np.float32), np.ones((1, D), np.float32)).astype(bf16)

    in_maps = []
    for c in range(8):
        b, g = c // 2, c % 2
        order = np.r_[g * NQ : (g + 1) * NQ, (1 - g) * NQ : (2 - g) * NQ]
        xTc = np.ascontiguousarray(x[b][order].T).astype(bf16)
        in_maps.append({"xT": xTc, "wqkvT": wqkvT, "woutT": woutT, "selIn": selmat})

    nc = _get_nc()
    LAST = run_bass_kernel_spmd(nc, in_maps, core_ids=list(range(8)))

    out = np.empty((B, N, DIM), dtype=np.float32)
    for c in range(8):
        b, g = c // 2, c % 2
        out[b, g * NQ : (g + 1) * NQ, :] = LAST.results[c]["outT"].T
    return out
